# revision 14
# baseline (speedup 1.0000x reference)
"""Causal self-attention (GPT-style block) on 8 Trainium2 NeuronCores.

Problem: x[4, 2048, 768], w_attn[2304, 768], b_attn[2304], w_proj[768, 768],
b_proj[768]; 12 heads of size 64; causal softmax attention; output [4, 2048, 768].

Sharding: batch x heads. core = 2*b + g handles batch b (of 4) and the 6 heads
g*6..g*6+5 (tensor parallel over heads). Each core:
  1. QKV projection for its head slice in bf16 (fp32 PSUM accumulation),
     producing Q^T/K^T in [r, t] layout and V in [t, r] layout (plus a fused
     ones column for softmax denominators). Biases are folded into the
     PSUM-evacuation adds (no bias matmuls).
  2. Flash-style causal attention per head, fused per q-block with the QKV
     chunk that unlocks it: S^T kv-tile PAIRS [128, 1024] via PE, one merged
     exp per pair on ACT (scale=1/8; columns outside the ragged diagonal
     writes hold stale-but-bounded PSUM values whose exp is never read),
     triangular mask on diagonal blocks via DVE, O^T accumulation on PE with
     the V-ones column yielding the softmax denominator for free.
     Normalization: DVE reciprocal, GpSimd partition_broadcast, DVE multiply.
     O matmuls and normalization are software-pipelined one pair behind the
     S/exp stream, carried across head and q-block boundaries.
  3. c_proj with its 384 local channels -> partial y[2048, 768]; proj(J) is
     emitted after QKV(J+1) so PE has independent work while the last head
     of block J normalizes.
  4. Host sums core pairs and adds b_proj.

All matmuls run in bf16 with fp32 PSUM accumulation.
"""
import os

import numpy as np
import ml_dtypes

os.environ.setdefault("JAX_COMPILATION_CACHE_DIR", "/tmp/jaxcache")
os.environ.setdefault("JAX_PERSISTENT_CACHE_MIN_COMPILE_TIME_SECS", "0")
os.environ.setdefault("JAX_PERSISTENT_CACHE_MIN_ENTRY_SIZE_BYTES", "0")

import concourse.bass as bass
import concourse.bacc as bacc
import concourse.tile as tile
from concourse import mybir
from concourse.bass_utils import run_bass_kernel_spmd

B, T, C, H = 4, 2048, 768, 12
HS = 64          # head size
HL = 6           # heads per core
CL = HL * HS     # 384 local channels per core
NQ = 512         # q block width
NCH = T // NQ    # 4 chunks
NKB = T // 128   # 16 kv blocks
NCORES = 8
F32 = mybir.dt.float32
BF16 = mybir.dt.bfloat16
EXP = mybir.ActivationFunctionType.Exp
BF = ml_dtypes.bfloat16


def build_bass(repeat=1):
    nc = bacc.Bacc(num_devices=NCORES)
    xT = nc.declare_dram_parameter("xT", [C, T], BF16, isOutput=False)
    wqkT = nc.declare_dram_parameter("wqkT", [C, 2 * CL], BF16, isOutput=False)
    wvT = nc.declare_dram_parameter("wvT", [C, CL], BF16, isOutput=False)
    wpT = nc.declare_dram_parameter("wpT", [CL, C], BF16, isOutput=False)
    bqk = nc.declare_dram_parameter("bqk", [128, 6], F32, isOutput=False)
    bv = nc.declare_dram_parameter("bv", [128, CL], F32, isOutput=False)
    tri = nc.declare_dram_parameter("tri", [128, 128], BF16, isOutput=False)
    ones = nc.declare_dram_parameter("ones", [128, NKB * HL], BF16, isOutput=False)
    y_out = nc.declare_dram_parameter("y_out", [T, C], F32, isOutput=True)

    with tile.TileContext(nc) as tc:
        with (
            tc.tile_pool(name="const", bufs=1) as constp,
            tc.tile_pool(name="wpool", bufs=1) as wpool,
            tc.tile_pool(name="qkv", bufs=1) as qkvp,
            tc.tile_pool(name="xch", bufs=2) as xchp,
            tc.tile_pool(name="ptp", bufs=3) as ptp,
            tc.tile_pool(name="small", bufs=2) as smallp,
            tc.tile_pool(name="otsb", bufs=4) as otsbp,
            tc.tile_pool(name="yev", bufs=2) as yevp,
            tc.tile_pool(name="ps_sp", bufs=2, space="PSUM") as spool,
            tc.tile_pool(name="ps_q", bufs=2, space="PSUM") as qpool,
            tc.tile_pool(name="ps_ot", bufs=2, space="PSUM") as opool,
        ):
            # ---- constants + weights ----
            tri_sb = constp.tile([128, 128], BF16)
            nc.sync.dma_start(out=tri_sb, in_=tri[:, :])
            bqk_sb = constp.tile([128, 6], F32)
            nc.scalar.dma_start(out=bqk_sb, in_=bqk[:, :])
            bv_sb = constp.tile([128, CL], F32)
            nc.scalar.dma_start(out=bv_sb, in_=bv[:, :])

            engs = [nc.sync, nc.gpsimd, nc.scalar]
            wqk_sb = []
            for cb in range(6):
                wt = wpool.tile([128, 2 * CL], BF16, tag=f"wqk{cb}")
                engs[cb % 3].dma_start(
                    out=wt, in_=wqkT[cb * 128:(cb + 1) * 128, :])
                wqk_sb.append(wt)
            wv_sb = []
            for cb in range(6):
                wt = wpool.tile([128, CL], BF16, tag=f"wv{cb}")
                engs[(cb + 1) % 3].dma_start(
                    out=wt, in_=wvT[cb * 128:(cb + 1) * 128, :])
                wv_sb.append(wt)
            wp_sb = []
            for cb in range(3):
                wt = wpool.tile([128, C], BF16, tag=f"wp{cb}")
                engs[(cb + 2) % 3].dma_start(
                    out=wt, in_=wpT[cb * 128:(cb + 1) * 128, :])
                wp_sb.append(wt)

            # persistent activations
            QT = [qkvp.tile([128, T], BF16, tag=f"qt{i}", name=f"qt{i}") for i in range(3)]
            KT = [qkvp.tile([128, T], BF16, tag=f"kt{i}", name=f"kt{i}") for i in range(3)]
            V = qkvp.tile([128, NKB, HL, HS + 1], BF16, tag="v")
            nc.sync.dma_start(
                out=V[:, :, :, HS],
                in_=ones[:, 0:NKB * HL].rearrange("p (a b) -> p a b", b=HL))

            xTr = xT[:, :].rearrange("(cb p) t -> p cb t", p=128)

            for _rep in range(repeat):
                phase_body(nc, tc, xTr, wqk_sb, wv_sb, wp_sb, bqk_sb, bv_sb,
                           tri_sb, QT, KT, V, y_out,
                           xchp, ptp, smallp, otsbp, yevp, spool, qpool, opool)
    nc.finalize()
    return nc


def phase_body(nc, tc, xTr, wqk_sb, wv_sb, wp_sb, bqk_sb, bv_sb, tri_sb,
               QT, KT, V, y_out,
               xchp, ptp, smallp, otsbp, yevp, spool, qpool, opool):
    engs = [nc.sync, nc.gpsimd, nc.scalar]
    state = {"o": None, "norm": None}

    def flush():
        # Emit the deferred O matmuls of the previous pair, then the
        # deferred normalization of the previous head (which reads the ot
        # those matmuls complete).
        if state["o"] is not None:
            for o in state["o"]:
                nc.tensor.matmul(o.pop("out"), **o)
            state["o"] = None
        if state["norm"] is not None:
            state["norm"]()
            state["norm"] = None

    def dma_chunk(J):
        xc = xchp.tile([128, 6, NQ], BF16, tag="xc")
        for cb in range(6):
            engs[(cb + J) % 3].dma_start(
                out=xc[:, cb, :], in_=xTr[:, cb, J * NQ:(J + 1) * NQ])
        return xc

    def qkv_units(J, xc, groups=None):
        """One closure per matmul of chunk J's QKV projection; the last
        closure of each accumulation group also emits the bias-folding
        PSUM evacuation. groups selects a subset (e.g. ['r0','v2'])."""
        qs = slice(J * NQ, (J + 1) * NQ)
        units = []
        if groups is None:
            groups = [f"r{rb}" for rb in range(6)] + [f"v{tb}" for tb in range(4)]
        for g in groups:
            kind, idx = g[0], int(g[1:])
            cell = {}
            for cb in range(6):
                def unit(cb=cb, kind=kind, idx=idx, cell=cell):
                    if "ps" not in cell:
                        cell["ps"] = qpool.tile([128, NQ], F32, tag="pq", name="pq")
                    ps = cell["ps"]
                    if kind == "r":
                        nc.tensor.matmul(
                            ps[:, 0:NQ],
                            lhsT=wqk_sb[cb][:, idx * 128:(idx + 1) * 128],
                            rhs=xc[:, cb, :], start=(cb == 0), stop=(cb == 5))
                        if cb == 5:
                            dst = QT[idx] if idx < 3 else KT[idx - 3]
                            nc.vector.tensor_add(
                                dst[:, qs], ps[:, 0:NQ],
                                bqk_sb[:, idx:idx + 1].broadcast_to([128, NQ]))
                    else:
                        nc.tensor.matmul(
                            ps[:, 0:CL],
                            lhsT=xc[:, cb, idx * 128:(idx + 1) * 128],
                            rhs=wv_sb[cb], start=(cb == 0), stop=(cb == 5))
                        if cb == 5:
                            nc.vector.tensor_add(
                                V[:, J * 4 + idx, :, 0:HS],
                                ps[:, 0:CL].rearrange("p (h d) -> p h d", d=HS),
                                bv_sb.rearrange("p (h d) -> p h d", d=HS))
                units.append(unit)
        return units

    def proj_units(J, ots):
        """One closure per matmul of c_proj for q-block J; the last closure
        of each half-group also emits the evacuation + store DMA."""
        units = []
        for i in range(4):
            cell = {}
            for half in range(2):
                for cb in range(3):
                    def unit(i=i, half=half, cb=cb, cell=cell):
                        if half not in cell:
                            cell[half] = qpool.tile([128, NQ], F32, tag="pq", name="pq")
                        yps = cell[half]
                        nc.tensor.matmul(
                            yps[:, 0:CL],
                            lhsT=ots[cb][:, i * 128:(i + 1) * 128],
                            rhs=wp_sb[cb][:, half * CL:(half + 1) * CL],
                            start=(cb == 0), stop=(cb == 2))
                        if cb == 2:
                            if "yt" not in cell:
                                cell["yt"] = yevp.tile([128, C], F32, tag="yt", name="yt")
                            yt = cell["yt"]
                            nc.vector.tensor_copy(
                                yt[:, half * CL:(half + 1) * CL],
                                yps[:, 0:CL])
                            if half == 1:
                                nc.sync.dma_start(
                                    out=y_out[(J * 4 + i) * 128:
                                              (J * 4 + i + 1) * 128, :],
                                    in_=yt)
                    units.append(unit)
        return units

    def emit_attn(J, filler):
        qs = slice(J * NQ, (J + 1) * NQ)
        ots = [otsbp.tile([128, NQ], BF16, tag=f"ots{cb}", name=f"ots{cb}")
               for cb in range(3)]
        total_pairs = HL * (2 * J + 2)
        pair_no = 0
        emitted = 0
        total_units = len(filler)
        for h in range(HL):
            kb, po = h // 2, (h % 2) * HS
            qt = QT[kb][po:po + HS, qs]
            ot = opool.tile([HS + 1, NQ], F32, tag="ot")
            npairs = 2 * J + 2
            for p in range(npairs):
                sp = spool.tile([128, 2 * NQ], F32, tag="sp")
                pt = ptp.tile([128, 2 * NQ], BF16, tag="pt")
                # S matmuls for the two kv tiles of this pair (ragged on
                # diagonal tiles)
                for i in (0, 1):
                    t = 2 * p + i
                    d = t - 4 * J
                    off = i * NQ
                    if d < 0:        # full kv tile
                        nc.tensor.matmul(
                            sp[:, off:off + NQ],
                            lhsT=KT[kb][po:po + HS, t * 128:(t + 1) * 128],
                            rhs=qt, start=True, stop=True)
                    else:            # diagonal tile: ragged width
                        qoff = 128 * d
                        nc.tensor.matmul(
                            sp[:, off + qoff:off + NQ],
                            lhsT=KT[kb][po:po + HS, t * 128:(t + 1) * 128],
                            rhs=QT[kb][po:po + HS, J * NQ + qoff:(J + 1) * NQ],
                            start=True, stop=True)
                # one exp over the whole pair; for the last (fully diagonal)
                # pair only 384 of 1024 columns are valid, so exp just the
                # two ragged regions instead
                if p == 2 * J + 1:
                    for i in (0, 1):
                        qoff = 128 * (2 * p + i - 4 * J)
                        off = i * NQ
                        nc.scalar.activation(pt[:, off + qoff:off + NQ],
                                             sp[:, off + qoff:off + NQ],
                                             EXP, scale=0.125)
                else:
                    nc.scalar.activation(pt, sp, EXP, scale=0.125)
                # triangular mask on diagonal 128-blocks
                for i in (0, 1):
                    t = 2 * p + i
                    d = t - 4 * J
                    if d >= 0:
                        off = i * NQ + 128 * d
                        nc.vector.tensor_mul(pt[:, off:off + 128],
                                             pt[:, off:off + 128], tri_sb)
                # O matmuls for this pair, deferred one step so PE streams
                # S(p+1) while ACT computes exp(p)
                omms = []
                for i in (0, 1):
                    t = 2 * p + i
                    d = t - 4 * J
                    off = i * NQ
                    last = (t == 4 * J + 3)
                    if d < 0:
                        omms.append(dict(out=ot, lhsT=V[:, t, h, :],
                                         rhs=pt[:, off:off + NQ],
                                         start=(t == 0), stop=last))
                    else:
                        qoff = 128 * d
                        omms.append(dict(out=ot[:, qoff:NQ],
                                         lhsT=V[:, t, h, :],
                                         rhs=pt[:, off + qoff:off + NQ],
                                         start=(t == 0), stop=last))
                flush()
                state["o"] = omms
                # spread filler matmuls (next chunk's QKV, previous block's
                # c_proj) through the attention stream to keep PE dense
                # while ACT owns the exp critical path
                pair_no += 1
                want = (total_units * pair_no) // total_pairs
                while emitted < want:
                    filler[emitted]()
                    emitted += 1
            # Defer the normalization of this head until after the next
            # head's (or q-block's) first S-pair, so PE is never starved
            # behind the exp -> O -> reciprocal chain.

            def norm(ot=ot, kb=kb, po=po, ots=ots):
                rec = smallp.tile([1, NQ], F32, tag="rec")
                nc.vector.reciprocal(rec, ot[HS:HS + 1, :])
                bcs = smallp.tile([HS, NQ], F32, tag="bcs")
                nc.gpsimd.partition_broadcast(bcs, rec)
                nc.vector.tensor_mul(ots[kb][po:po + HS, :], ot[0:HS, :], bcs)
            state["norm"] = norm
        # drain leftover filler
        for u in filler[emitted:]:
            u()
        return ots

    # Prologue: chunk 0 DMA + the QKV groups attention block 0 needs first
    xc0 = dma_chunk(0)
    for u in qkv_units(0, xc0, ["r0", "r3"] + [f"v{tb}" for tb in range(4)]):
        u()
    xcs = {0: xc0}
    all_ots = []
    for J in range(NCH):
        # Filler is budgeted against each block's ACT-vs-PE deficit: the
        # last block (largest exp volume) absorbs every deferred c_proj.
        filler = []
        if J == 0:
            filler.extend(qkv_units(0, xc0, ["r1", "r4", "r2", "r5"]))
        if J + 1 < NCH:
            xcs[J + 1] = dma_chunk(J + 1)
            filler.extend(qkv_units(J + 1, xcs[J + 1]))
        if J == NCH - 1:
            for Jp in range(NCH - 1):
                filler.extend(proj_units(Jp, all_ots[Jp]))
        all_ots.append(emit_attn(J, filler))
    flush()
    for u in proj_units(NCH - 1, all_ots[NCH - 1]):
        u()


def make_in_maps(x, w_attn, b_attn, w_proj):
    x = np.asarray(x, dtype=np.float32)
    w_attn = np.asarray(w_attn, dtype=np.float32)
    b_attn = np.asarray(b_attn, dtype=np.float32)
    w_proj = np.asarray(w_proj, dtype=np.float32)
    # valid iff kv <= q with kv on partitions (rows), q on free dim (cols)
    tri = np.triu(np.ones((128, 128), dtype=BF))
    ones = np.ones((128, NKB * HL), dtype=BF)
    in_maps = []
    for core in range(NCORES):
        b, g = divmod(core, 2)
        sl = slice(g * CL, (g + 1) * CL)
        wq, wk, wv = (w_attn[i * C:(i + 1) * C][sl] for i in range(3))
        bq, bk, bv_ = (b_attn[i * C:(i + 1) * C][sl] for i in range(3))
        bqk = np.concatenate([bq, bk])                      # [768]
        in_maps.append({
            "xT": np.ascontiguousarray(x[b].T).astype(BF),
            "wqkT": np.ascontiguousarray(np.concatenate([wq, wk], 0).T).astype(BF),
            "wvT": np.ascontiguousarray(wv.T).astype(BF),
            "wpT": np.ascontiguousarray(w_proj[:, sl].T).astype(BF),
            "bqk": np.ascontiguousarray(bqk.reshape(6, 128).T).copy(),
            "bv": np.broadcast_to(bv_[None, :], (128, CL)).copy(),
            "tri": tri,
            "ones": ones,
        })
    return in_maps


def assemble(results, b_proj):
    out = np.empty((B, T, C), dtype=np.float32)
    for b in range(B):
        out[b] = results[2 * b]["y_out"] + results[2 * b + 1]["y_out"]
    out += np.asarray(b_proj, dtype=np.float32)[None, None, :]
    return out


_CACHE = {}


def _get_nc():
    if "nc" not in _CACHE:
        _CACHE["nc"] = build_bass()
    return _CACHE["nc"]


def kernel(x, w_attn, b_attn, w_proj, b_proj):
    in_maps = make_in_maps(x, w_attn, b_attn, w_proj)
    res = run_bass_kernel_spmd(_get_nc(), in_maps, list(range(NCORES)))
    return assemble(res.results, b_proj)


# revision 15
# speedup vs baseline: 1.2369x; 1.2369x over previous
"""Causal self-attention (GPT-style block) on 8 Trainium2 NeuronCores.

Problem: x[4, 2048, 768], w_attn[2304, 768], b_attn[2304], w_proj[768, 768],
b_proj[768]; 12 heads of size 64; causal softmax attention; output [4, 2048, 768].

Sharding: batch x heads. core = 2*b + g handles batch b (of 4) and the 6 heads
g*6..g*6+5 (tensor parallel over heads). Each core:
  1. QKV projection for its head slice in bf16 (fp32 PSUM accumulation),
     producing Q^T/K^T in [r, t] layout and V in [t, r] layout (plus a fused
     ones column for softmax denominators). Biases are folded into the
     PSUM-evacuation adds (no bias matmuls).
  2. Flash-style causal attention per head, fused per q-block with the QKV
     chunk that unlocks it: S^T kv-tile PAIRS [128, 1024] via PE, one merged
     exp per pair on ACT (scale=1/8; columns outside the ragged diagonal
     writes hold stale-but-bounded PSUM values whose exp is never read),
     triangular mask on diagonal blocks via DVE, O^T accumulation on PE with
     the V-ones column yielding the softmax denominator for free.
     Normalization: DVE reciprocal, GpSimd partition_broadcast, DVE multiply.
     O matmuls and normalization are software-pipelined one pair behind the
     S/exp stream, carried across head and q-block boundaries.
  3. c_proj with its 384 local channels -> partial y[2048, 768]; proj(J) is
     emitted after QKV(J+1) so PE has independent work while the last head
     of block J normalizes.
  4. Host sums core pairs and adds b_proj.

All matmuls run in bf16 with fp32 PSUM accumulation.
"""
import os

import numpy as np
import ml_dtypes

os.environ.setdefault("JAX_COMPILATION_CACHE_DIR", "/tmp/jaxcache")
os.environ.setdefault("JAX_PERSISTENT_CACHE_MIN_COMPILE_TIME_SECS", "0")
os.environ.setdefault("JAX_PERSISTENT_CACHE_MIN_ENTRY_SIZE_BYTES", "0")

import concourse.bass as bass
import concourse.bacc as bacc
import concourse.tile as tile
from concourse import mybir
from concourse.bass_utils import run_bass_kernel_spmd

B, T, C, H = 4, 2048, 768, 12
HS = 64          # head size
HL = 6           # heads per core
CL = HL * HS     # 384 local channels per core
NQ = 512         # q block width
NCH = T // NQ    # 4 chunks
NKB = T // 128   # 16 kv blocks
NCORES = 8
F32 = mybir.dt.float32
BF16 = mybir.dt.bfloat16
EXP = mybir.ActivationFunctionType.Exp
BF = ml_dtypes.bfloat16


def build_bass(repeat=1):
    nc = bacc.Bacc(num_devices=NCORES)
    xT = nc.declare_dram_parameter("xT", [C, T], BF16, isOutput=False)
    wqkT = nc.declare_dram_parameter("wqkT", [C, 2 * CL], BF16, isOutput=False)
    wvT = nc.declare_dram_parameter("wvT", [C, CL], BF16, isOutput=False)
    wpT = nc.declare_dram_parameter("wpT", [CL, C], BF16, isOutput=False)
    bqk = nc.declare_dram_parameter("bqk", [128, 6], F32, isOutput=False)
    bv = nc.declare_dram_parameter("bv", [128, CL], F32, isOutput=False)
    tri = nc.declare_dram_parameter("tri", [128, 128], BF16, isOutput=False)
    ones = nc.declare_dram_parameter("ones", [128, NKB * HL], BF16, isOutput=False)
    y_out = nc.declare_dram_parameter("y_out", [T, C], F32, isOutput=True)

    with tile.TileContext(nc) as tc:
        with (
            tc.tile_pool(name="const", bufs=1) as constp,
            tc.tile_pool(name="wpool", bufs=1) as wpool,
            tc.tile_pool(name="qkv", bufs=1) as qkvp,
            tc.tile_pool(name="xch", bufs=3) as xchp,
            tc.tile_pool(name="ptp", bufs=4) as ptp,
            tc.tile_pool(name="small", bufs=3) as smallp,
            tc.tile_pool(name="otsb", bufs=4) as otsbp,
            tc.tile_pool(name="yev", bufs=3) as yevp,
            tc.tile_pool(name="ps_sp", bufs=2, space="PSUM") as spool,
            tc.tile_pool(name="ps_q", bufs=2, space="PSUM") as qpool,
            tc.tile_pool(name="ps_ot", bufs=2, space="PSUM") as opool,
        ):
            # ---- constants + weights ----
            tri_sb = constp.tile([128, 128], BF16)
            nc.sync.dma_start(out=tri_sb, in_=tri[:, :])
            bqk_sb = constp.tile([128, 6], F32)
            nc.scalar.dma_start(out=bqk_sb, in_=bqk[:, :])
            bv_sb = constp.tile([128, CL], F32)
            nc.scalar.dma_start(out=bv_sb, in_=bv[:, :])

            engs = [nc.sync, nc.gpsimd, nc.scalar]
            wqk_sb = []
            for cb in range(6):
                wt = wpool.tile([128, 2 * CL], BF16, tag=f"wqk{cb}")
                engs[cb % 3].dma_start(
                    out=wt, in_=wqkT[cb * 128:(cb + 1) * 128, :])
                wqk_sb.append(wt)
            wv_sb = []
            for cb in range(6):
                wt = wpool.tile([128, CL], BF16, tag=f"wv{cb}")
                engs[(cb + 1) % 3].dma_start(
                    out=wt, in_=wvT[cb * 128:(cb + 1) * 128, :])
                wv_sb.append(wt)
            wp_sb = []
            for cb in range(3):
                wt = wpool.tile([128, C], BF16, tag=f"wp{cb}")
                engs[(cb + 2) % 3].dma_start(
                    out=wt, in_=wpT[cb * 128:(cb + 1) * 128, :])
                wp_sb.append(wt)

            # persistent activations
            QT = [qkvp.tile([128, T], BF16, tag=f"qt{i}", name=f"qt{i}") for i in range(3)]
            KT = [qkvp.tile([128, T], BF16, tag=f"kt{i}", name=f"kt{i}") for i in range(3)]
            V = qkvp.tile([128, NKB, HL, HS + 1], BF16, tag="v")
            nc.sync.dma_start(
                out=V[:, :, :, HS],
                in_=ones[:, 0:NKB * HL].rearrange("p (a b) -> p a b", b=HL))

            xTr = xT[:, :].rearrange("(cb p) t -> p cb t", p=128)

            for _rep in range(repeat):
                phase_body(nc, tc, xTr, wqk_sb, wv_sb, wp_sb, bqk_sb, bv_sb,
                           tri_sb, QT, KT, V, y_out,
                           xchp, ptp, smallp, otsbp, yevp, spool, qpool, opool)
    nc.finalize()
    return nc


def phase_body(nc, tc, xTr, wqk_sb, wv_sb, wp_sb, bqk_sb, bv_sb, tri_sb,
               QT, KT, V, y_out,
               xchp, ptp, smallp, otsbp, yevp, spool, qpool, opool):
    engs = [nc.sync, nc.gpsimd, nc.scalar]
    state = {"o": None, "norm": None}

    def flush():
        # Emit the deferred O matmuls of the previous pair, then the
        # deferred normalization of the previous head (which reads the ot
        # those matmuls complete).
        if state["o"] is not None:
            for o in state["o"]:
                nc.tensor.matmul(o.pop("out"), **o)
            state["o"] = None
        if state["norm"] is not None:
            state["norm"]()
            state["norm"] = None

    def dma_chunk(J):
        xc = xchp.tile([128, 6, NQ], BF16, tag="xc")
        for cb in range(6):
            engs[(cb + J) % 3].dma_start(
                out=xc[:, cb, :], in_=xTr[:, cb, J * NQ:(J + 1) * NQ])
        return xc

    def qkv_units(J, xc, groups=None):
        """One closure per matmul of chunk J's QKV projection; the last
        closure of each accumulation group also emits the bias-folding
        PSUM evacuation. groups selects a subset (e.g. ['r0','v2'])."""
        qs = slice(J * NQ, (J + 1) * NQ)
        units = []
        if groups is None:
            groups = [f"r{rb}" for rb in range(6)] + [f"v{tb}" for tb in range(4)]
        for g in groups:
            kind, idx = g[0], int(g[1:])
            cell = {}
            for cb in range(6):
                def unit(cb=cb, kind=kind, idx=idx, cell=cell):
                    if "ps" not in cell:
                        cell["ps"] = qpool.tile([128, NQ], F32, tag="pq", name="pq")
                    ps = cell["ps"]
                    if kind == "r":
                        nc.tensor.matmul(
                            ps[:, 0:NQ],
                            lhsT=wqk_sb[cb][:, idx * 128:(idx + 1) * 128],
                            rhs=xc[:, cb, :], start=(cb == 0), stop=(cb == 5))
                        if cb == 5:
                            dst = QT[idx] if idx < 3 else KT[idx - 3]
                            nc.vector.tensor_add(
                                dst[:, qs], ps[:, 0:NQ],
                                bqk_sb[:, idx:idx + 1].broadcast_to([128, NQ]))
                    else:
                        nc.tensor.matmul(
                            ps[:, 0:CL],
                            lhsT=xc[:, cb, idx * 128:(idx + 1) * 128],
                            rhs=wv_sb[cb], start=(cb == 0), stop=(cb == 5))
                        if cb == 5:
                            nc.vector.tensor_add(
                                V[:, J * 4 + idx, :, 0:HS],
                                ps[:, 0:CL].rearrange("p (h d) -> p h d", d=HS),
                                bv_sb.rearrange("p (h d) -> p h d", d=HS))
                units.append(unit)
        return units

    def proj_units(J, ots):
        """One closure per matmul of c_proj for q-block J; the last closure
        of each half-group also emits the evacuation + store DMA."""
        units = []
        for i in range(4):
            cell = {}
            for half in range(2):
                for cb in range(3):
                    def unit(i=i, half=half, cb=cb, cell=cell):
                        if half not in cell:
                            cell[half] = qpool.tile([128, NQ], F32, tag="pq", name="pq")
                        yps = cell[half]
                        nc.tensor.matmul(
                            yps[:, 0:CL],
                            lhsT=ots[cb][:, i * 128:(i + 1) * 128],
                            rhs=wp_sb[cb][:, half * CL:(half + 1) * CL],
                            start=(cb == 0), stop=(cb == 2))
                        if cb == 2:
                            if "yt" not in cell:
                                cell["yt"] = yevp.tile([128, C], F32, tag="yt", name="yt")
                            yt = cell["yt"]
                            nc.vector.tensor_copy(
                                yt[:, half * CL:(half + 1) * CL],
                                yps[:, 0:CL])
                            if half == 1:
                                nc.sync.dma_start(
                                    out=y_out[(J * 4 + i) * 128:
                                              (J * 4 + i + 1) * 128, :],
                                    in_=yt)
                    units.append(unit)
        return units

    def emit_attn(J, filler):
        qs = slice(J * NQ, (J + 1) * NQ)
        ots = [otsbp.tile([128, NQ], BF16, tag=f"ots{cb}", name=f"ots{cb}")
               for cb in range(3)]
        total_pairs = HL * (2 * J + 2)
        pair_no = 0
        emitted = 0
        total_units = len(filler)
        for h in range(HL):
            kb, po = h // 2, (h % 2) * HS
            qt = QT[kb][po:po + HS, qs]
            ot = opool.tile([HS + 1, NQ], F32, tag="ot")
            npairs = 2 * J + 2
            for p in range(npairs):
                sp = spool.tile([128, 2 * NQ], F32, tag="sp")
                pt = ptp.tile([128, 2 * NQ], BF16, tag="pt")
                # S matmuls for the two kv tiles of this pair (ragged on
                # diagonal tiles)
                for i in (0, 1):
                    t = 2 * p + i
                    d = t - 4 * J
                    off = i * NQ
                    if d < 0:        # full kv tile
                        nc.tensor.matmul(
                            sp[:, off:off + NQ],
                            lhsT=KT[kb][po:po + HS, t * 128:(t + 1) * 128],
                            rhs=qt, start=True, stop=True)
                    else:            # diagonal tile: ragged width
                        qoff = 128 * d
                        nc.tensor.matmul(
                            sp[:, off + qoff:off + NQ],
                            lhsT=KT[kb][po:po + HS, t * 128:(t + 1) * 128],
                            rhs=QT[kb][po:po + HS, J * NQ + qoff:(J + 1) * NQ],
                            start=True, stop=True)
                # one exp over the whole pair; for the last (fully diagonal)
                # pair only 384 of 1024 columns are valid, so exp just the
                # two ragged regions instead
                if p == 2 * J + 1:
                    for i in (0, 1):
                        qoff = 128 * (2 * p + i - 4 * J)
                        off = i * NQ
                        nc.scalar.activation(pt[:, off + qoff:off + NQ],
                                             sp[:, off + qoff:off + NQ],
                                             EXP, scale=0.125)
                else:
                    nc.scalar.activation(pt, sp, EXP, scale=0.125)
                # triangular mask on diagonal 128-blocks
                for i in (0, 1):
                    t = 2 * p + i
                    d = t - 4 * J
                    if d >= 0:
                        off = i * NQ + 128 * d
                        nc.vector.tensor_mul(pt[:, off:off + 128],
                                             pt[:, off:off + 128], tri_sb)
                # O matmuls for this pair, deferred one step so PE streams
                # S(p+1) while ACT computes exp(p)
                omms = []
                for i in (0, 1):
                    t = 2 * p + i
                    d = t - 4 * J
                    off = i * NQ
                    last = (t == 4 * J + 3)
                    if d < 0:
                        omms.append(dict(out=ot, lhsT=V[:, t, h, :],
                                         rhs=pt[:, off:off + NQ],
                                         start=(t == 0), stop=last))
                    else:
                        qoff = 128 * d
                        omms.append(dict(out=ot[:, qoff:NQ],
                                         lhsT=V[:, t, h, :],
                                         rhs=pt[:, off + qoff:off + NQ],
                                         start=(t == 0), stop=last))
                flush()
                state["o"] = omms
                # spread filler matmuls (next chunk's QKV, previous block's
                # c_proj) through the attention stream to keep PE dense
                # while ACT owns the exp critical path
                pair_no += 1
                want = (total_units * pair_no) // total_pairs
                while emitted < want:
                    filler[emitted]()
                    emitted += 1
            # Defer the normalization of this head until after the next
            # head's (or q-block's) first S-pair, so PE is never starved
            # behind the exp -> O -> reciprocal chain.

            def norm(ot=ot, kb=kb, po=po, ots=ots):
                rec = smallp.tile([1, NQ], F32, tag="rec")
                nc.vector.reciprocal(rec, ot[HS:HS + 1, :])
                bcs = smallp.tile([HS, NQ], F32, tag="bcs")
                nc.gpsimd.partition_broadcast(bcs, rec)
                nc.vector.tensor_mul(ots[kb][po:po + HS, :], ot[0:HS, :], bcs)
            state["norm"] = norm
        # drain leftover filler
        for u in filler[emitted:]:
            u()
        return ots

    # Prologue: chunk 0 DMA + the QKV groups attention block 0 needs first
    xc0 = dma_chunk(0)
    for u in qkv_units(0, xc0, ["r0", "r3"] + [f"v{tb}" for tb in range(4)]):
        u()
    xcs = {0: xc0}
    all_ots = []
    for J in range(NCH):
        # Filler is budgeted against each block's ACT-vs-PE deficit: the
        # last block (largest exp volume) absorbs every deferred c_proj.
        filler = []
        if J == 0:
            filler.extend(qkv_units(0, xc0, ["r1", "r4", "r2", "r5"]))
        if J + 1 < NCH:
            xcs[J + 1] = dma_chunk(J + 1)
            filler.extend(qkv_units(J + 1, xcs[J + 1]))
        if J == NCH - 1:
            for Jp in range(NCH - 1):
                filler.extend(proj_units(Jp, all_ots[Jp]))
        all_ots.append(emit_attn(J, filler))
    flush()
    for u in proj_units(NCH - 1, all_ots[NCH - 1]):
        u()


def make_in_maps(x, w_attn, b_attn, w_proj):
    x = np.asarray(x, dtype=np.float32)
    w_attn = np.asarray(w_attn, dtype=np.float32)
    b_attn = np.asarray(b_attn, dtype=np.float32)
    w_proj = np.asarray(w_proj, dtype=np.float32)
    # valid iff kv <= q with kv on partitions (rows), q on free dim (cols)
    tri = np.triu(np.ones((128, 128), dtype=BF))
    ones = np.ones((128, NKB * HL), dtype=BF)
    in_maps = []
    for core in range(NCORES):
        b, g = divmod(core, 2)
        sl = slice(g * CL, (g + 1) * CL)
        wq, wk, wv = (w_attn[i * C:(i + 1) * C][sl] for i in range(3))
        bq, bk, bv_ = (b_attn[i * C:(i + 1) * C][sl] for i in range(3))
        bqk = np.concatenate([bq, bk])                      # [768]
        in_maps.append({
            "xT": np.ascontiguousarray(x[b].T).astype(BF),
            "wqkT": np.ascontiguousarray(np.concatenate([wq, wk], 0).T).astype(BF),
            "wvT": np.ascontiguousarray(wv.T).astype(BF),
            "wpT": np.ascontiguousarray(w_proj[:, sl].T).astype(BF),
            "bqk": np.ascontiguousarray(bqk.reshape(6, 128).T).copy(),
            "bv": np.broadcast_to(bv_[None, :], (128, CL)).copy(),
            "tri": tri,
            "ones": ones,
        })
    return in_maps


def assemble(results, b_proj):
    out = np.empty((B, T, C), dtype=np.float32)
    for b in range(B):
        out[b] = results[2 * b]["y_out"] + results[2 * b + 1]["y_out"]
    out += np.asarray(b_proj, dtype=np.float32)[None, None, :]
    return out


_CACHE = {}


def _get_nc():
    if "nc" not in _CACHE:
        _CACHE["nc"] = build_bass()
    return _CACHE["nc"]


def kernel(x, w_attn, b_attn, w_proj, b_proj):
    in_maps = make_in_maps(x, w_attn, b_attn, w_proj)
    res = run_bass_kernel_spmd(_get_nc(), in_maps, list(range(NCORES)))
    return assemble(res.results, b_proj)
